# revision 33
# baseline (speedup 1.0000x reference)
"""Gaussian falloff vortex-velocity kernel for Trainium2 (8 NeuronCores).

Math: out[b,h,w,:] = sum_n tau_n * exp(-r2/sig_n^2) / sqrt(r2) * (d2, -d1)
with d1 = py - y_n, d2 = px - x_n, r2 = d1^2 + d2^2.

Device pipeline per (pair of 512-point tiles, 128-particle block):
  TensorE : -r2 via a 12-row fp16 matmul (each fp32 factor split into exact
            fp16 hi+lo halves; fp16xfp16 products are exact in the fp32 PSUM
            accumulate, so precision matches an fp32 matmul at 1 cyc/row)
  ScalarE : y0i = Identity(bits(-r2) * -0.5 + bias_n)   (quake rsqrt seed;
            bias_n = MAGIC + 2^22*log2(s2_n) folds the /s2 scale in)
            e   = Exp(iv_n * -r2) -> fp16                (iv_n = 1/s2_n)
  VectorE : q = Halley-polish rsqrt custom DVE op -> fp16
            (w2 = (1.25*iv_n)*(-r2)*y0^2; q = y0*(1.875 + w2*(1 + 0.24*w2)))
  V/Pool  : g = e * q  (fp16 tensor_tensor; spread over both engines)
  TensorE : S[3] += W^T g (fp16 matmul over particle blocks)
  where rsqrt(r2) = rsqrt(u)/|s| is folded into W = tau*{1,x,y}/|s|.
Finalize: out_u = px*S0 - S1, out_v = S2 - py*S0 (via DRAM re-layout).

Sharding: grid rows H split across 8 cores (32 rows each); particles
replicated.
"""

import sys

import numpy as np

B, H, W, N = 2, 256, 256, 512
NCORES = 8
HPC = H // NCORES          # 32 rows per core
PPB = HPC * W              # 8192 points per batch per core
NT = PPB // 512            # 16 point-tiles of 512 per batch
NK = N // 128              # 4 particle blocks
NR = 12                    # split-matmul rows

MAGIC2 = float(0x5F3759DF) - 0.5 * 2.0**31

_cache = {}


def _bass_modules():
    if "/opt/trn_rl_repo" not in sys.path:
        sys.path.insert(0, "/opt/trn_rl_repo")
    import concourse.bass as bass
    import concourse.mybir as mybir
    import concourse.tile as tile
    from concourse import bacc, dve_ops
    from concourse.bass_utils import run_bass_kernel_spmd

    return bass, mybir, tile, run_bass_kernel_spmd, bacc, dve_ops


def _halley_ref(in0, in1, s0, s1, imm2):
    w2 = (in1 * s0) * (in0 * in0)
    return in0 * (s1 + w2 * (1.0 + imm2 * w2))


def _register_halley():
    _, _, _, _, _, dve_ops = _bass_modules()
    from concourse.dve_spec import (
        C0, C1, C2, One, Spec, Src0, Src1, _has_src1, lower, sq,
    )
    from concourse.dve_uop import DveOpSpec

    name = "RSQRT_HALLEY_SCALED_ANT"
    if name in dve_ops._SUB_OPCODE_FOR_NAME:
        return next(o for o in dve_ops.OPS if o.name == name)
    w2 = (Src1 * C0) * sq(Src0)
    spec = Spec(body=Src0 * (C1 + w2 * (One + C2 * w2)), reference=_halley_ref)
    row = max(dve_ops._SUB_OPCODE_FOR_NAME.values()) + 1
    assert row < 0x20
    dve_ops._SUB_OPCODE_FOR_NAME[name] = row
    shas = {}
    for ver in ("v3", "v4"):
        s = DveOpSpec(
            name=name, opcode=row, uops=lower(spec, ver=ver),
            rd1_en=_has_src1(spec),
        )
        shas[ver] = s.sha(ver)
    op = dve_ops.DveOp(name, spec, subdim=False, uops_sha=shas)
    dve_ops.OPS.append(op)
    dve_ops.CUSTOM_DVE_SPECS[name] = spec
    return op


def _build_nc(repeat=1):
    bass, mybir, tile, _, bacc, _ = _bass_modules()
    halley = _register_halley()
    f32 = mybir.dt.float32
    f16 = mybir.dt.float16
    i32 = mybir.dt.int32
    AF = mybir.ActivationFunctionType
    ALU = mybir.AluOpType

    nc = bacc.Bacc(None)
    stu_d = nc.declare_dram_parameter("stu", [NR, B * NK * 128], f16, isOutput=False)
    ws_d = nc.declare_dram_parameter("ws", [128, B * NK * 3], f16, isOutput=False)
    ppar_d = nc.declare_dram_parameter("ppar", [128, B * NK * 3], f32, isOutput=False)
    mvp_d = nc.declare_dram_parameter("mvp", [B, NT, NR, 512], f16, isOutput=False)
    ptsf_d = nc.declare_dram_parameter("ptsf", [B, 2, 128, PPB // 128], f32, isOutput=False)
    out_d = nc.declare_dram_parameter("out", [B, 2, 128, PPB // 128], f32, isOutput=True)

    with tile.TileContext(nc) as tc:
        with (
            tc.tile_pool(name="const", bufs=1) as cpool,
            tc.tile_pool(name="mv", bufs=4) as mvpool,
            tc.tile_pool(name="temps", bufs=4) as temps,
            tc.tile_pool(name="psu", bufs=3, space=bass.MemorySpace.PSUM) as psu,
            tc.tile_pool(name="psacc", bufs=1, space=bass.MemorySpace.PSUM) as psacc,
            tc.tile_pool(name="fin", bufs=2) as fin,
            tc.tile_pool(name="dscratch", bufs=1, space="DRAM") as dpool,
        ):
            stu = cpool.tile([NR, B * NK * 128], f16)
            nc.sync.dma_start(stu[:], stu_d[:])
            ws = cpool.tile([128, B * NK * 3], f16)
            nc.sync.dma_start(ws[:], ws_d[:])
            # per-particle columns: [iv, seed_bias, 1.25*iv] per (b,k)
            ppar = cpool.tile([128, B * NK * 3], f32)
            nc.sync.dma_start(ppar[:], ppar_d[:])
            scratch = dpool.tile([B, 3, PPB], f32)

            srs = scratch[:].rearrange("b three (p f) -> b three p f", p=128)

            def finalize(b):
                s0 = fin.tile([128, PPB // 128], f32, tag="s0")
                nc.sync.dma_start(s0[:], srs[b, 0])
                s1 = fin.tile([128, PPB // 128], f32, tag="s1")
                nc.sync.dma_start(s1[:], srs[b, 1])
                s2 = fin.tile([128, PPB // 128], f32, tag="s2")
                nc.sync.dma_start(s2[:], srs[b, 2])
                pyf = fin.tile([128, PPB // 128], f32, tag="pyf")
                nc.sync.dma_start(pyf[:], ptsf_d[b, 0])
                pxf = fin.tile([128, PPB // 128], f32, tag="pxf")
                nc.sync.dma_start(pxf[:], ptsf_d[b, 1])
                tu = fin.tile([128, PPB // 128], f32, tag="tu")
                nc.vector.tensor_mul(tu[:], pxf[:], s0[:])
                uo = fin.tile([128, PPB // 128], f32, tag="uo")
                nc.vector.tensor_sub(uo[:], tu[:], s1[:])
                tv = fin.tile([128, PPB // 128], f32, tag="tv")
                nc.vector.tensor_mul(tv[:], pyf[:], s0[:])
                vo = fin.tile([128, PPB // 128], f32, tag="vo")
                nc.vector.tensor_sub(vo[:], s2[:], tv[:])
                nc.sync.dma_start(out_d[b, 0], uo[:])
                nc.sync.dma_start(out_d[b, 1], vo[:])

            def pcol(idx, blk):
                return ppar[:, blk * 3 + idx:blk * 3 + idx + 1]

            for rep in range(repeat):
              for b in range(B):
                for TP in range(NT // 2):   # pairs of point-tiles
                    if rep == repeat - 1 and b == 1 and TP == 1:
                        finalize(0)   # overlap batch-0 tail with batch-1 work
                    mv = mvpool.tile([NR, 1024], f16, tag="mv")
                    nc.sync.dma_start(mv[:, 0:512], mvp_d[b, 2 * TP])
                    nc.sync.dma_start(mv[:, 512:1024], mvp_d[b, 2 * TP + 1])
                    sacc = psacc.tile([3, 1024], f32, tag="sacc")

                    def combine_and_reduce(prev):
                        e0, q0, k0, gi0 = prev
                        g = temps.tile([128, 1024], f16, tag="g")
                        if gi0 % 4 == 2:
                            nc.vector.tensor_mul(g[:], e0[:], q0[:])
                        else:
                            nc.gpsimd.tensor_mul(g[:], e0[:], q0[:])
                        w3 = (b * NK + k0) * 3
                        nc.tensor.matmul(
                            sacc[:, 0:512], ws[:, w3:w3 + 3], g[:, 0:512],
                            start=(k0 == 0), stop=(k0 == NK - 1),
                        )
                        nc.tensor.matmul(
                            sacc[:, 512:1024], ws[:, w3:w3 + 3],
                            g[:, 512:1024],
                            start=(k0 == 0), stop=(k0 == NK - 1),
                        )

                    prev = None
                    for k in range(NK):      # one particle block per group
                        blk = b * NK + k
                        c = blk * 128
                        psU = psu.tile([128, 1024], f32, tag="psU")
                        nc.tensor.matmul(
                            psU[:, 0:512], stu[:, c:c + 128], mv[:, 0:512],
                            start=True, stop=True,
                        )
                        nc.tensor.matmul(
                            psU[:, 512:1024], stu[:, c:c + 128],
                            mv[:, 512:1024],
                            start=True, stop=True,
                        )
                        y0i = temps.tile([128, 1024], i32, tag="y0i")
                        gi = (b * (NT // 2) + TP) * NK + k
                        if gi % 4 in (0, 3):
                            nc.vector.tensor_scalar(
                                y0i[:], psU[:].bitcast(i32),
                                -0.5, pcol(1, blk),
                                ALU.mult, ALU.add,
                            )
                        else:
                            nc.scalar.activation(
                                y0i[:], psU[:].bitcast(i32), AF.Identity,
                                bias=pcol(1, blk), scale=-0.5,
                            )
                        e = temps.tile([128, 1024], f16, tag="e")
                        nc.scalar.activation(
                            e[:], psU[:], AF.Exp, scale=pcol(0, blk),
                        )
                        if prev is not None:
                            combine_and_reduce(prev)
                        q = temps.tile([128, 1024], f16, tag="q")
                        nc.vector._custom_dve(
                            halley, out=q[:], in0=y0i[:].bitcast(f32),
                            in1=psU[:], s0=pcol(2, blk), s1=1.875, imm2=0.24,
                        )
                        prev = (e, q, k, gi)
                    combine_and_reduce(prev)
                    srow = temps.tile([3, 1024], f32, tag="srow")
                    nc.scalar.copy(srow[:], sacc[:])
                    nc.sync.dma_start(
                        scratch[b, :, (2 * TP) * 512:(2 * TP + 2) * 512],
                        srow[:],
                    )

            finalize(1)
    nc.compile()
    return nc


def _split16(a):
    hi = a.astype(np.float16)
    lo = (a - hi.astype(np.float32)).astype(np.float16)
    return hi, lo


def _prep_inputs(vortex_feature, points):
    vf = np.asarray(vortex_feature, dtype=np.float32)
    pts_full = np.asarray(points, dtype=np.float32)

    y = vf[:, :, 0]
    x = vf[:, :, 1]
    tau = vf[:, :, 2]
    sig = vf[:, :, 3]
    s2 = sig * sig
    abss = np.abs(sig)

    yh, yl = _split16(2.0 * y)      # [B, N] fp16 each
    xh, xl = _split16(2.0 * x)
    c = -(y * y + x * x)
    ch, cl = _split16(c)
    ones = np.ones_like(y, dtype=np.float16)
    # stationary rows pair with moving rows:
    # mv: [pyh, pyl, pyh, pyl, pxh, pxl, pxh, pxl, 1, 1, p2h, p2l]
    # st: [yh,  yh,  yl,  yl,  xh,  xh,  xl,  xl,  ch, cl, -1,  -1 ]
    st_rows = np.stack([yh, yh, yl, yl, xh, xh, xl, xl, ch, cl,
                        -ones, -ones])              # [12, B, N] fp16
    stu = np.ascontiguousarray(st_rows.reshape(NR, B * N)).astype(np.float16)

    w3 = np.stack([tau / abss, tau * x / abss, tau * y / abss], axis=-1)
    ws = np.ascontiguousarray(
        w3.reshape(B, NK, 128, 3).transpose(2, 0, 1, 3).reshape(128, B * NK * 3)
    ).astype(np.float16)

    iv = (1.0 / s2).astype(np.float32)
    bias = (MAGIC2 + 2.0**22 * np.log2(s2)).astype(np.float32)
    c0h = (1.25 * iv).astype(np.float32)
    pp = np.stack([iv, bias, c0h], axis=-1)        # [B, N, 3]
    ppar = np.ascontiguousarray(
        pp.reshape(B, NK, 128, 3).transpose(2, 0, 1, 3).reshape(128, B * NK * 3)
    ).astype(np.float32)

    in_maps = []
    for i in range(NCORES):
        sl = pts_full[:, i * HPC:(i + 1) * HPC]            # [B, 32, 256, 2]
        flat = sl.reshape(B, PPB, 2)
        pts = np.ascontiguousarray(flat.transpose(0, 2, 1))  # [B, 2, PPB]
        ptsf = np.ascontiguousarray(pts.reshape(B, 2, 128, PPB // 128))
        py = pts[:, 0].reshape(B, NT, 512).astype(np.float32)
        px = pts[:, 1].reshape(B, NT, 512).astype(np.float32)
        pyh, pyl = _split16(py)
        pxh, pxl = _split16(px)
        p2 = py * py + px * px
        p2h, p2l = _split16(p2)
        one = np.ones_like(pyh)
        mvp = np.stack(
            [pyh, pyl, pyh, pyl, pxh, pxl, pxh, pxl, one, one, p2h, p2l],
            axis=2,
        ).astype(np.float16)                               # [B, NT, 12, 512]
        in_maps.append({"stu": stu, "ws": ws, "ppar": ppar,
                        "mvp": mvp, "ptsf": ptsf})
    return in_maps


def _assemble(results):
    out = np.zeros((B, H, W, 2), dtype=np.float32)
    for i in range(NCORES):
        o = np.asarray(results[i]["out"])  # [B, 2, 128, PPB//128]
        o = o.reshape(B, 2, PPB).transpose(0, 2, 1).reshape(B, HPC, W, 2)
        out[:, i * HPC:(i + 1) * HPC] = o
    return out


def _run(vortex_feature, points, trace=False):
    _, _, _, run_bass_kernel_spmd, _b, _ = _bass_modules()
    if "nc" not in _cache:
        _cache["nc"] = _build_nc()
    in_maps = _prep_inputs(vortex_feature, points)
    res = run_bass_kernel_spmd(
        _cache["nc"], in_maps, list(range(NCORES)), trace=trace
    )
    return _assemble(res.results), res


def kernel(vortex_feature, points):
    out, _ = _run(vortex_feature, points, trace=False)
    return out
